# revision 47
# baseline (speedup 1.0000x reference)
"""Linear-attention MultiHeadAttentionBlock kernel for 8 Trainium2 NeuronCores.

Sharding: core c handles (batch b = c//2, head-group g = c%2).  Each core
computes, for its batch's q/k/v and its 8 heads (512 of the 1024 d_model
dims), entirely in bf16 matmuls (fp32 PSUM accumulation):

    Kf   = elu(X_k @ Wk_g^T)+1            (n-space, 16 tiles)
    ksT8 = sum_n Kf                       (PE accumulation, d'-partition)
    Vp   = X_v @ Wv_g^T                   (n-space)
    kvsT = Vp^T @ Kf  per d'-chunk        (PE accumulation over n)
    KVW  = blockdiag-masked kvsT @ WoS^T  (fused out-projection)
    per n-chunk (512 columns), software-pipelined:
      QfT  = elu(Wq_g @ X_q^T)+1          (transposed space: d' on partitions)
      ZpreT= KS32^T @ QfT  -> zra = 1/ZpreT      (head-major (8, n))
      QzT  = QfT * (sel8^T @ zra)         (zr broadcast via PE matmul)
      yT  += KVW^T @ QzT                  (final projection, bf16 out)
Host sums the two per-batch partials in fp32.

Phase order K -> V -> Q keeps the Z-normalization chain off the critical
path (it software-pipelines per 512-column n-chunk against the Q-projection
and the final GEMM).  The K phase opens with a 7-deep kc-major window so
the PE tracks DMA chunk arrival instead of stalling on a full contraction.
bf16 halves HBM traffic vs fp32 (the fp32 baseline was DMA-starved for its
first ~25us); rel. error stays ~4e-3 << the 2e-2 gate.
"""

import numpy as np

import concourse.bass as bass
import concourse.mybir as mybir
import concourse.tile as tile
from concourse import bacc
from concourse.bass_utils import run_bass_kernel_spmd

P = 128
L = 2048          # sequence length
DM = 1024         # d_model
DG = 512          # per-core head-group width (8 heads x 64)
NT = L // P       # 16 n-tiles
KC = DM // P      # 8 contraction chunks
DT = DG // P      # 4 d'-chunks (2 heads each)
NCH = 4           # n-chunks of 512
BF16 = mybir.dt.bfloat16
F32 = mybir.dt.float32
F32R = mybir.dt.float32r

_CACHE = {}


def build_nc(repeats=1):
    nc = bacc.Bacc(None, target_bir_lowering=False)

    xq_d = nc.dram_tensor("xqT", [DM, L], BF16, kind="ExternalInput")
    xk_d = nc.dram_tensor("xkT", [DM, L], BF16, kind="ExternalInput")
    xv_d = nc.dram_tensor("xvT", [DM, L], BF16, kind="ExternalInput")
    wq_d = nc.dram_tensor("wqT", [DM, DG], BF16, kind="ExternalInput")
    wk_d = nc.dram_tensor("wkT", [DM, DG], BF16, kind="ExternalInput")
    wv_d = nc.dram_tensor("wvT", [DM, DG], BF16, kind="ExternalInput")
    wo_d = nc.dram_tensor("woT", [DG, DM], BF16, kind="ExternalInput")
    sel_d = nc.dram_tensor("sel8", [32, P], F32R, kind="ExternalInput")
    y_d = nc.dram_tensor("y", [DM, L], BF16, kind="ExternalOutput")

    with tile.TileContext(nc) as tc:
        with (
            nc.allow_low_precision(
                reason="bf16 elementwise outputs; rel-err gate is 2e-2"),
            tc.tile_pool(name="const", bufs=1) as cpool,
            tc.tile_pool(name="xkv", bufs=16) as xkv,    # (128,2048) x chunks
            tc.tile_pool(name="xq", bufs=8) as xqp,      # (128,2048) resident
            tc.tile_pool(name="wt", bufs=16) as wt,      # wk/wv chunks
            tc.tile_pool(name="wq", bufs=8) as wqp,      # wq persistent
            tc.tile_pool(name="wo", bufs=4) as wop,      # (128,1024) w_o chunks
            tc.tile_pool(name="kf", bufs=16) as kfp,     # Kf persistent
            tc.tile_pool(name="qft", bufs=8) as qftp,    # QfT rotating
            tc.tile_pool(name="vp", bufs=4) as vpp,
            tc.tile_pool(name="tmp", bufs=8) as tmp,
            tc.tile_pool(name="misc", bufs=1) as misc,
            tc.tile_pool(name="ysb", bufs=6) as ysb,
            tc.tile_pool(name="pp", bufs=4, space="PSUM") as pp,
            tc.tile_pool(name="kvp", bufs=4, space="PSUM") as kvp,
        ):
            ones_f = cpool.tile([P, 2], F32, name="ones_f")
            nc.gpsimd.memset(ones_f[:], 1.0)
            ones2t = cpool.tile([P, 2], BF16, name="ones2")
            nc.vector.tensor_copy(ones2t[:], ones_f[:])
            ones2 = ones2t[:]
            sel8 = []
            for _dt in range(DT):
                st = cpool.tile([8, P], F32R, name=f"sel8_{_dt}")
                nc.sync.dma_start(st[:], sel_d[_dt * 8:(_dt + 1) * 8, :])
                sel8.append(st)
            for _rep in range(repeats):
                body(nc, tc, ones2, sel8,
                     xkv, xqp, wt, wqp, wop, kfp, qftp, vpp, tmp, misc, ysb,
                     pp, kvp,
                     xq_d, xk_d, xv_d, wq_d, wk_d, wv_d, wo_d, y_d)

    nc.compile()
    return nc


def body(nc, tc, ones2, sel8,
         xkv, xqp, wt, wqp, wop, kfp, qftp, vpp, tmp, misc, ysb, pp, kvp,
         xq_d, xk_d, xv_d, wq_d, wk_d, wv_d, wo_d, y_d):
    Exp = mybir.ActivationFunctionType.Exp
    Relu = mybir.ActivationFunctionType.Relu
    Alu = mybir.AluOpType

    def feature_map(ps, dst):
        # dst = elu(ps)+1 = exp(min(ps,0)) + relu(ps)
        t0 = tmp.tile([P, DG], F32, tag="tmp", name="t0")
        t1 = tmp.tile([P, DG], F32, tag="tmp", name="t1")
        nc.vector.tensor_scalar(t0[:], ps[:], 0.0, None, Alu.min)
        nc.scalar.activation(t1[:], ps[:], Relu)
        nc.scalar.activation(dst[:], t0[:], Exp)
        nc.vector.tensor_tensor(dst[:], dst[:], t1[:], Alu.add)

    def dma_x_half(src_d, half, kc, tag):
        # x chunks ride the HWDGE ring (SP) in half-rows: small enough to
        # arrive at PE cadence, big enough to stay transfer-bound.
        t = xkv.tile([P, L // 2], BF16, tag=tag, name="xh", bufs=16)
        nc.sync.dma_start(
            t[:], src_d[kc * P:(kc + 1) * P,
                        half * (L // 2):(half + 1) * (L // 2)])
        return t

    # ---------------- input DMAs (single SP ring: FIFO arrival order
    # matches consumption; cross-ring scheduling reorders transfers badly)
    wk = []
    xkh = [[None] * KC, [None] * KC]
    xk0q = []
    for q4 in range(2):
        t = xkv.tile([P, DG], BF16, tag="xk0", name="xk0q", bufs=2)
        nc.sync.dma_start(t[:], xk_d[0:P, q4 * DG:(q4 + 1) * DG])
        xk0q.append(t)
    for kc in range(KC):
        if kc == 0:
            wt_t = wt.tile([P, DG], BF16, tag="wt")
            nc.sync.dma_start(wt_t[:], wk_d[0:P, :])
            wk.append(wt_t)
            continue
        xkh[0][kc] = dma_x_half(xk_d, 0, kc, "xk")
        wt_t = wt.tile([P, DG], BF16, tag="wt")
        nc.sync.dma_start(wt_t[:], wk_d[kc * P:(kc + 1) * P, :])
        wk.append(wt_t)
    for kc in range(KC):
        xkh[1][kc] = dma_x_half(xk_d, 1, kc, "xk")

    wv = []
    xvh = [[None] * KC, [None] * KC]
    for kc in range(KC):
        xvh[0][kc] = dma_x_half(xv_d, 0, kc, "xv")
        wt_t = wt.tile([P, DG], BF16, tag="wt")
        nc.sync.dma_start(wt_t[:], wv_d[kc * P:(kc + 1) * P, :])
        wv.append(wt_t)
    for kc in range(KC):
        xvh[1][kc] = dma_x_half(xv_d, 1, kc, "xv")

    wo = []
    for dc in range(DT):
        wo_t = wop.tile([P, DM], BF16, tag="wo", name="wo_t")
        nc.sync.dma_start(wo_t[:], wo_d[dc * P:(dc + 1) * P, :])
        wo.append(wo_t)
    wq = []
    for kc in range(KC):
        wq_t = wqp.tile([P, DG], BF16, tag="wq")
        nc.sync.dma_start(wq_t[:], wq_d[kc * P:(kc + 1) * P, :])
        wq.append(wq_t)
    xq = []
    for kc in range(KC):
        xq_t = xqp.tile([P, L], BF16, tag="xq")
        nc.sync.dma_start(xq_t[:], xq_d[kc * P:(kc + 1) * P, :])
        xq.append(xq_t)

    # ---------------- Phase K: Kf (n-space) + ksum ----------------
    # PE warmup: the HAM clock gate holds the PE at 1.2 GHz until ~3.4us of
    # sustained activity.  Burn the ramp on dummy matmuls over the tiny sel8
    # constants while the first x/w chunks are still in flight.
    for _w in range(7):
        wps = kvp.tile([P, P], F32, tag="acc", name="warm")
        nc.tensor.matmul(wps[:], sel8[_w % 4][:], sel8[(_w + 1) % 4][:],
                         start=True, stop=True)

    kf = [None] * NT
    # Startup window: 7 groups kc-major so the PE tracks chunk arrival.
    WIN = 7
    pss = []
    for j in range(WIN):
        pool, tag = (pp, "pp") if j < 4 else (kvp, "acc")
        pss.append(pool.tile([P, DG], F32, tag=tag, name=f"ksu{j}"))
    ksT8 = kvp.tile([P, 8], F32, tag="acc", name="ksT8")
    def xk_slice(kc, nt):
        if kc == 0:
            if nt < 8:
                return xk0q[nt // 4][:, (nt % 4) * P:(nt % 4 + 1) * P]
            return xkh[1][0][:, (nt - 8) * P:(nt - 7) * P]
        half, sub = nt // 8, nt % 8
        return xkh[half][kc][:, sub * P:(sub + 1) * P]

    for kc in range(KC):
        for j in range(WIN):
            nc.tensor.matmul(
                pss[j][:],
                xk_slice(kc, j),
                wk[kc][:],
                start=(kc == 0), stop=(kc == KC - 1),
            )

    def ksum_mm(kft, nt):
        # one PSUM bank, 4 interleaved column groups: start clears the
        # whole bank, so only the very first matmul may carry start=True
        for dt in range(DT):
            nc.tensor.matmul(
                ksT8[:, dt * 2:dt * 2 + 2],
                kft[:, dt * P:(dt + 1) * P],
                ones2,
                start=(nt == 0 and dt == 0),
                stop=(nt == NT - 1 and dt == DT - 1),
                skip_group_check=True,
            )

    for j in range(WIN):
        kft = kfp.tile([P, DG], BF16, tag="kf")
        feature_map(pss[j], kft)
        kf[j] = kft
        ksum_mm(kft, j)
    for nt in range(WIN, NT):
        ps = pp.tile([P, DG], F32, tag="pp")
        for kc in range(KC):
            nc.tensor.matmul(
                ps[:],
                xk_slice(kc, nt),
                wk[kc][:],
                start=(kc == 0), stop=(kc == KC - 1),
            )
        kft = kfp.tile([P, DG], BF16, tag="kf")
        feature_map(ps, kft)
        kf[nt] = kft
        ksum_mm(kft, nt)

    # KS32: (128, 32) bf16; block dt holds ksum for its 2 heads in
    # columns 2dt (rows 0:64) and 2dt+1 (rows 64:128), zeros elsewhere.
    z32 = tmp.tile([P, 32], F32, tag="z32", bufs=1)
    nc.gpsimd.memset(z32[:], 0.0)
    ks32 = misc.tile([P, 32], BF16, tag="ks32", bufs=1)
    nc.vector.tensor_copy(ks32[:], z32[:])
    for dt in range(DT):
        c = dt * 8 + 2 * dt
        nc.scalar.copy(ks32[0:64, c:c + 1], ksT8[0:64, dt * 2:dt * 2 + 1])
        nc.scalar.copy(ks32[64:128, c + 1:c + 2],
                       ksT8[64:128, dt * 2:dt * 2 + 1])

    # ---------------- Phase V: Vp + kvsT accumulation ----------------
    # kvsT[dt] (128 dv, 128 dk) = Vp_dt^T @ Kf_dt  (= KV^T for the pair)
    kvsT = [kvp.tile([P, P], F32, tag="acc", name=f"kvsT{_d}")
            for _d in range(DT)]
    for nt in range(NT):
        half, sub = nt // 8, nt % 8
        ps = pp.tile([P, DG], F32, tag="pp")
        for kc in range(KC):
            nc.tensor.matmul(
                ps[:],
                xvh[half][kc][:, sub * P:(sub + 1) * P],
                wv[kc][:],
                start=(kc == 0), stop=(kc == KC - 1),
            )
        vp_t = vpp.tile([P, DG], BF16, tag="vp")
        if nt % 2 == 0:
            nc.vector.tensor_copy(vp_t[:], ps[:])
        else:
            nc.scalar.copy(vp_t[:], ps[:])
        for dt in range(DT):
            nc.tensor.matmul(
                kvsT[dt][:],
                vp_t[:, dt * P:(dt + 1) * P],
                kf[nt][:, dt * P:(dt + 1) * P],
                start=(nt == 0), stop=(nt == NT - 1),
            )

    # ---------------- Phase Q + Z + final, software-pipelined ----------
    zcat = tmp.tile([P, P], F32, tag="zcat", bufs=1)
    nc.gpsimd.memset(zcat[:], 0.0)
    kvw = []

    def emit_kvw():
        # KVW: fused (blockdiag KV) @ WoS^T, emitted under Q-proj's shadow
        for dt in range(DT):
            kcat = misc.tile([P, P], BF16, tag="kcat", bufs=4)
            nc.vector.tensor_copy(kcat[:], zcat[:])
            nc.scalar.copy(kcat[0:64, 0:64], kvsT[dt][0:64, 0:64])
            nc.scalar.copy(kcat[64:128, 64:128], kvsT[dt][64:128, 64:128])
            kvw_t = misc.tile([P, DM], BF16, tag="kvw", bufs=4)
            for h2 in range(2):
                psk = pp.tile([P, DG], F32, tag="pp")
                nc.tensor.matmul(
                    psk[:], kcat[:], wo[dt][:, h2 * DG:(h2 + 1) * DG],
                    start=True, stop=True,
                )
                if h2 == 0:
                    nc.vector.tensor_copy(kvw_t[:, 0:DG], psk[:])
                else:
                    nc.scalar.copy(kvw_t[:, DG:DM], psk[:])
            kvw.append(kvw_t)

    def emit_proj_q(nch):
        qz = []
        for dt in range(DT):
            ps = pp.tile([P, DG], F32, tag="pp")
            for kc in range(KC):
                nc.tensor.matmul(
                    ps[:],
                    wq[kc][:, dt * P:(dt + 1) * P],
                    xq[kc][:, nch * DG:(nch + 1) * DG],
                    start=(kc == 0), stop=(kc == KC - 1),
                )
            qf = qftp.tile([P, DG], BF16, tag="qft")
            feature_map(ps, qf)
            qz.append(qf)
        return qz

    def emit_zchain(qz):
        # ZpreT (8 heads, 512 n); zra = 1/ZpreT; QzT = QfT * selbcast(zra)
        zp = kvp.tile([8, DG], F32, tag="acc", name="zp")
        for dt in range(DT):
            nc.tensor.matmul(
                zp[:], ks32[:, dt * 8:(dt + 1) * 8], qz[dt][:],
                start=(dt == 0), stop=(dt == DT - 1),
            )
        zra = misc.tile([8, DG], F32R, tag="zra", bufs=2)
        nc.vector.reciprocal(zra[:], zp[:])
        for dt in range(DT):
            zrp = kvp.tile([P, DG], F32, tag="acc", name="zrp")
            nc.tensor.matmul(zrp[:], sel8[dt][:], zra[:],
                             start=True, stop=True)
            nc.vector.tensor_tensor(qz[dt][:], qz[dt][:], zrp[:], Alu.mult)

    def emit_ygemm(nch, qz):
        for jb in range(8):
            yps = pp.tile([P, DG], F32, tag="pp")
            for dt in range(DT):
                nc.tensor.matmul(
                    yps[:],
                    kvw[dt][:, jb * P:(jb + 1) * P],
                    qz[dt][:],
                    start=(dt == 0), stop=(dt == DT - 1),
                )
            yt = ysb.tile([P, DG], BF16, tag="ysb")
            if (jb + nch) % 2 == 0:
                nc.vector.tensor_copy(yt[:], yps[:])
            else:
                nc.scalar.copy(yt[:], yps[:])
            nc.sync.dma_start(
                y_d[jb * P:(jb + 1) * P, nch * DG:(nch + 1) * DG], yt[:])

    prev = None
    for nch in range(NCH):
        qz = emit_proj_q(nch)
        if nch == 0:
            emit_kvw()
        emit_zchain(qz)
        if prev is not None:
            emit_ygemm(*prev)
        prev = (nch, qz)
    emit_ygemm(*prev)


def make_in_maps(q, k, v, w_q, w_k, w_v, w_o):
    bf16 = mybir.dt.np(BF16)
    f32 = np.float32
    q = np.asarray(q, dtype=f32)
    k = np.asarray(k, dtype=f32)
    v = np.asarray(v, dtype=f32)
    w_q = np.asarray(w_q, dtype=f32)
    w_k = np.asarray(w_k, dtype=f32)
    w_v = np.asarray(w_v, dtype=f32)
    w_o = np.asarray(w_o, dtype=f32)
    B = q.shape[0]
    xqT = [q[b].T.astype(bf16) for b in range(B)]
    xkT = [k[b].T.astype(bf16) for b in range(B)]
    xvT = [v[b].T.astype(bf16) for b in range(B)]
    wqT = [w_q[g * DG:(g + 1) * DG, :].T.astype(bf16) for g in range(2)]
    wkT = [w_k[g * DG:(g + 1) * DG, :].T.astype(bf16) for g in range(2)]
    wvT = [w_v[g * DG:(g + 1) * DG, :].T.astype(bf16) for g in range(2)]
    woT = [w_o[:, g * DG:(g + 1) * DG].T.astype(bf16) for g in range(2)]
    sel8 = np.zeros((32, P), dtype=np.float32)
    for dt in range(4):
        sel8[dt * 8 + 2 * dt, 0:64] = 1.0
        sel8[dt * 8 + 2 * dt + 1, 64:128] = 1.0
    in_maps = []
    for c in range(8):
        b, g = c // 2, c % 2
        in_maps.append({
            "xqT": xqT[b], "xkT": xkT[b], "xvT": xvT[b],
            "wqT": wqT[g], "wkT": wkT[g], "wvT": wvT[g], "woT": woT[g],
            "sel8": sel8,
        })
    return in_maps


def kernel(q, k, v, mask, w_q, w_k, w_v, w_o):
    if "nc" not in _CACHE:
        _CACHE["nc"] = build_nc()
    nc = _CACHE["nc"]
    in_maps = make_in_maps(q, k, v, w_q, w_k, w_v, w_o)
    res = run_bass_kernel_spmd(nc, in_maps, list(range(8)))
    _CACHE["last_results"] = res
    B = np.asarray(q).shape[0]
    out = np.empty((B, L, DM), dtype=np.float32)
    for b in range(B):
        y0 = np.asarray(res.results[2 * b]["y"]).astype(np.float32)
        y1 = np.asarray(res.results[2 * b + 1]["y"]).astype(np.float32)
        out[b] = (y0 + y1).T
    return out


# revision 60
# speedup vs baseline: 1.0091x; 1.0091x over previous
"""Linear-attention MultiHeadAttentionBlock kernel for 8 Trainium2 NeuronCores.

Sharding: core c handles (batch b = c//2, head-group g = c%2).  Each core
computes, for its batch's q/k/v and its 8 heads (512 of the 1024 d_model
dims), entirely in bf16 matmuls (fp32 PSUM accumulation):

    Kf   = elu(X_k @ Wk_g^T)+1            (n-space, 16 tiles)
    ksT8 = sum_n Kf                       (PE accumulation, d'-partition)
    Vp   = X_v @ Wv_g^T                   (n-space)
    kvsT = Vp^T @ Kf  per d'-chunk        (PE accumulation over n)
    KVW  = blockdiag-masked kvsT @ WoS^T  (fused out-projection)
    per n-chunk (512 columns), software-pipelined:
      QfT  = elu(Wq_g @ X_q^T)+1          (transposed space: d' on partitions)
      ZpreT= KS32^T @ QfT  -> zra = 1/ZpreT      (head-major (8, n))
      QzT  = QfT * (sel8^T @ zra)         (zr broadcast via PE matmul)
      yT  += KVW^T @ QzT                  (final projection, bf16 out)
Host sums the two per-batch partials in fp32.

Phase order K -> V -> Q keeps the Z-normalization chain off the critical
path (it software-pipelines per 512-column n-chunk against the Q-projection
and the final GEMM).  The K phase opens with a 7-deep kc-major window so
the PE tracks DMA chunk arrival instead of stalling on a full contraction.
bf16 halves HBM traffic vs fp32 (the fp32 baseline was DMA-starved for its
first ~25us); rel. error stays ~4e-3 << the 2e-2 gate.
"""

import numpy as np

import concourse.bass as bass
import concourse.mybir as mybir
import concourse.tile as tile
from concourse import bacc
from concourse.bass_utils import run_bass_kernel_spmd

P = 128
L = 2048          # sequence length
DM = 1024         # d_model
DG = 512          # per-core head-group width (8 heads x 64)
NT = L // P       # 16 n-tiles
KC = DM // P      # 8 contraction chunks
DT = DG // P      # 4 d'-chunks (2 heads each)
NCH = 4           # n-chunks of 512
BF16 = mybir.dt.bfloat16
F32 = mybir.dt.float32
F32R = mybir.dt.float32r

_CACHE = {}


def build_nc(repeats=1):
    nc = bacc.Bacc(None, target_bir_lowering=False)

    xq_d = nc.dram_tensor("xqT", [DM, L], BF16, kind="ExternalInput")
    xk_d = nc.dram_tensor("xkT", [DM, L], BF16, kind="ExternalInput")
    xv_d = nc.dram_tensor("xvT", [DM, L], BF16, kind="ExternalInput")
    wq_d = nc.dram_tensor("wqT", [DM, DG], BF16, kind="ExternalInput")
    wk_d = nc.dram_tensor("wkT", [DM, DG], BF16, kind="ExternalInput")
    wv_d = nc.dram_tensor("wvT", [DM, DG], BF16, kind="ExternalInput")
    wo_d = nc.dram_tensor("woT", [DG, DM], BF16, kind="ExternalInput")
    sel_d = nc.dram_tensor("sel8", [32, P], F32R, kind="ExternalInput")
    y_d = nc.dram_tensor("y", [DM, L], BF16, kind="ExternalOutput")

    with tile.TileContext(nc) as tc:
        with (
            nc.allow_low_precision(
                reason="bf16 elementwise outputs; rel-err gate is 2e-2"),
            tc.tile_pool(name="const", bufs=1) as cpool,
            tc.tile_pool(name="xkv", bufs=16) as xkv,    # (128,2048) x chunks
            tc.tile_pool(name="xq", bufs=8) as xqp,      # (128,2048) resident
            tc.tile_pool(name="wt", bufs=16) as wt,      # wk/wv chunks
            tc.tile_pool(name="wq", bufs=8) as wqp,      # wq persistent
            tc.tile_pool(name="wo", bufs=4) as wop,      # (128,1024) w_o chunks
            tc.tile_pool(name="kf", bufs=16) as kfp,     # Kf persistent
            tc.tile_pool(name="qft", bufs=8) as qftp,    # QfT rotating
            tc.tile_pool(name="vp", bufs=4) as vpp,
            tc.tile_pool(name="tmp", bufs=8) as tmp,
            tc.tile_pool(name="misc", bufs=1) as misc,
            tc.tile_pool(name="ysb", bufs=6) as ysb,
            tc.tile_pool(name="pp", bufs=4, space="PSUM") as pp,
            tc.tile_pool(name="kvp", bufs=4, space="PSUM") as kvp,
        ):
            ones_f = cpool.tile([P, 2], F32, name="ones_f")
            nc.gpsimd.memset(ones_f[:], 1.0)
            ones2t = cpool.tile([P, 2], BF16, name="ones2")
            nc.vector.tensor_copy(ones2t[:], ones_f[:])
            ones2 = ones2t[:]
            sel8 = []
            for _dt in range(DT):
                st = cpool.tile([8, P], F32R, name=f"sel8_{_dt}")
                sel8.append(st)
            for _rep in range(repeats):
                body(nc, tc, ones2, sel8, sel_d,
                     xkv, xqp, wt, wqp, wop, kfp, qftp, vpp, tmp, misc, ysb,
                     pp, kvp,
                     xq_d, xk_d, xv_d, wq_d, wk_d, wv_d, wo_d, y_d)

    nc.compile()
    return nc


def body(nc, tc, ones2, sel8, sel_d,
         xkv, xqp, wt, wqp, wop, kfp, qftp, vpp, tmp, misc, ysb, pp, kvp,
         xq_d, xk_d, xv_d, wq_d, wk_d, wv_d, wo_d, y_d):
    Exp = mybir.ActivationFunctionType.Exp
    Relu = mybir.ActivationFunctionType.Relu
    Alu = mybir.AluOpType

    def feature_map(ps, dst, add_eng=None):
        # dst = elu(ps)+1 = exp(min(ps,0)) + relu(ps)
        t0 = tmp.tile([P, DG], F32, tag="tmp", name="t0")
        t1 = tmp.tile([P, DG], F32, tag="tmp", name="t1")
        nc.vector.tensor_scalar(t0[:], ps[:], 0.0, None, Alu.min)
        nc.scalar.activation(t1[:], ps[:], Relu)
        nc.scalar.activation(dst[:], t0[:], Exp)
        (add_eng or nc.vector).tensor_tensor(dst[:], dst[:], t1[:], Alu.add)

    def dma_x_half(src_d, half, kc, tag):
        # x chunks ride the HWDGE ring (SP) in half-rows: small enough to
        # arrive at PE cadence, big enough to stay transfer-bound.
        t = xkv.tile([P, L // 2], BF16, tag=tag, name="xh", bufs=16)
        nc.sync.dma_start(
            t[:], src_d[kc * P:(kc + 1) * P,
                        half * (L // 2):(half + 1) * (L // 2)])
        return t

    # ---------------- input DMAs (single SP ring: FIFO arrival order
    # matches consumption; cross-ring scheduling reorders transfers badly)
    wk = []
    xkh = [[None] * KC, [None] * KC]
    xk0q = []
    for q4 in range(2):
        t = xkv.tile([P, DG], BF16, tag="xk0", name="xk0q", bufs=2)
        nc.sync.dma_start(t[:], xk_d[0:P, q4 * DG:(q4 + 1) * DG])
        xk0q.append(t)
    for kc in range(KC):
        if kc == 0:
            wt_t = wt.tile([P, DG], BF16, tag="wt")
            nc.sync.dma_start(wt_t[:], wk_d[0:P, :])
            wk.append(wt_t)
            continue
        xkh[0][kc] = dma_x_half(xk_d, 0, kc, "xk")
        wt_t = wt.tile([P, DG], BF16, tag="wt")
        nc.sync.dma_start(wt_t[:], wk_d[kc * P:(kc + 1) * P, :])
        wk.append(wt_t)
    for kc in range(KC):
        xkh[1][kc] = dma_x_half(xk_d, 1, kc, "xk")

    wv = []
    xvh = [[None] * KC, [None] * KC]
    for kc in range(KC):
        xvh[0][kc] = dma_x_half(xv_d, 0, kc, "xv")
        wt_t = wt.tile([P, DG], BF16, tag="wt")
        nc.sync.dma_start(wt_t[:], wv_d[kc * P:(kc + 1) * P, :])
        wv.append(wt_t)
    for kc in range(KC):
        xvh[1][kc] = dma_x_half(xv_d, 1, kc, "xv")

    for _dt in range(DT):
        nc.sync.dma_start(sel8[_dt][:], sel_d[_dt * 8:(_dt + 1) * 8, :])
    wo = []
    for dc in range(DT):
        wo_t = wop.tile([P, DM], BF16, tag="wo", name="wo_t")
        nc.sync.dma_start(wo_t[:], wo_d[dc * P:(dc + 1) * P, :])
        wo.append(wo_t)
    wq = []
    for kc in range(KC):
        wq_t = wqp.tile([P, DG], BF16, tag="wq")
        nc.sync.dma_start(wq_t[:], wq_d[kc * P:(kc + 1) * P, :])
        wq.append(wq_t)
    xq = []
    for kc in range(KC):
        xq_t = xqp.tile([P, L], BF16, tag="xq")
        nc.sync.dma_start(xq_t[:], xq_d[kc * P:(kc + 1) * P, :])
        xq.append(xq_t)

    # ---------------- Phase K: Kf (n-space) + ksum ----------------
    kf = [None] * NT
    # Startup window: 7 groups kc-major so the PE tracks chunk arrival.
    WIN = 6
    pss = []
    for j in range(WIN):
        pool, tag = (pp, "pp") if j < 4 else (kvp, "acc")
        pss.append(pool.tile([P, DG], F32, tag=tag, name=f"ksu{j}"))
    ksT8 = kvp.tile([P, 8], F32, tag="acc", name="ksT8")
    def xk_slice(kc, nt):
        if kc == 0:
            if nt < 8:
                return xk0q[nt // 4][:, (nt % 4) * P:(nt % 4 + 1) * P]
            return xkh[1][0][:, (nt - 8) * P:(nt - 7) * P]
        half, sub = nt // 8, nt % 8
        return xkh[half][kc][:, sub * P:(sub + 1) * P]

    for kc in range(KC):
        for j in range(WIN):
            nc.tensor.matmul(
                pss[j][:],
                xk_slice(kc, j),
                wk[kc][:],
                start=(kc == 0), stop=(kc == KC - 1),
            )

    def ksum_mm(kft, nt):
        # one PSUM bank, 4 interleaved column groups: start clears the
        # whole bank, so only the very first matmul may carry start=True
        for dt in range(DT):
            nc.tensor.matmul(
                ksT8[:, dt * 2:dt * 2 + 2],
                kft[:, dt * P:(dt + 1) * P],
                ones2,
                start=(nt == 0 and dt == 0),
                stop=(nt == NT - 1 and dt == DT - 1),
                skip_group_check=True,
            )

    for j in range(WIN):
        kft = kfp.tile([P, DG], BF16, tag="kf")
        feature_map(pss[j], kft, add_eng=nc.gpsimd)
        kf[j] = kft
        ksum_mm(kft, j)
    for nt in range(WIN, NT):
        ps = pp.tile([P, DG], F32, tag="pp")
        for kc in range(KC):
            nc.tensor.matmul(
                ps[:],
                xk_slice(kc, nt),
                wk[kc][:],
                start=(kc == 0), stop=(kc == KC - 1),
            )
        kft = kfp.tile([P, DG], BF16, tag="kf")
        feature_map(ps, kft)
        kf[nt] = kft
        ksum_mm(kft, nt)

    # KS32: (128, 32) bf16; block dt holds ksum for its 2 heads in
    # columns 2dt (rows 0:64) and 2dt+1 (rows 64:128), zeros elsewhere.
    z32 = tmp.tile([P, 32], F32, tag="z32", bufs=1)
    nc.gpsimd.memset(z32[:], 0.0)
    ks32 = misc.tile([P, 32], BF16, tag="ks32", bufs=1)
    nc.vector.tensor_copy(ks32[:], z32[:])
    for dt in range(DT):
        c = dt * 8 + 2 * dt
        nc.scalar.copy(ks32[0:64, c:c + 1], ksT8[0:64, dt * 2:dt * 2 + 1])
        nc.scalar.copy(ks32[64:128, c + 1:c + 2],
                       ksT8[64:128, dt * 2:dt * 2 + 1])

    # ---------------- Phase V: Vp + kvsT accumulation ----------------
    # kvsT[dt] (128 dv, 128 dk) = Vp_dt^T @ Kf_dt  (= KV^T for the pair)
    kvsT = [kvp.tile([P, P], F32, tag="acc", name=f"kvsT{_d}")
            for _d in range(DT)]
    for nt in range(NT):
        half, sub = nt // 8, nt % 8
        ps = pp.tile([P, DG], F32, tag="pp")
        for kc in range(KC):
            nc.tensor.matmul(
                ps[:],
                xvh[half][kc][:, sub * P:(sub + 1) * P],
                wv[kc][:],
                start=(kc == 0), stop=(kc == KC - 1),
            )
        vp_t = vpp.tile([P, DG], BF16, tag="vp")
        if nt % 2 == 0:
            nc.vector.tensor_copy(vp_t[:], ps[:])
        else:
            nc.scalar.copy(vp_t[:], ps[:])
        for dt in range(DT):
            nc.tensor.matmul(
                kvsT[dt][:],
                vp_t[:, dt * P:(dt + 1) * P],
                kf[nt][:, dt * P:(dt + 1) * P],
                start=(nt == 0), stop=(nt == NT - 1),
            )

    # ---------------- Phase Q + Z + final, software-pipelined ----------
    zcat = tmp.tile([P, P], F32, tag="zcat", bufs=1)
    nc.gpsimd.memset(zcat[:], 0.0)
    kvw = []

    def emit_kvw():
        # KVW: fused (blockdiag KV) @ WoS^T, emitted under Q-proj's shadow
        for dt in range(DT):
            kcat = misc.tile([P, P], BF16, tag="kcat", bufs=4)
            nc.vector.tensor_copy(kcat[:], zcat[:])
            nc.scalar.copy(kcat[0:64, 0:64], kvsT[dt][0:64, 0:64])
            nc.scalar.copy(kcat[64:128, 64:128], kvsT[dt][64:128, 64:128])
            kvw_t = misc.tile([P, DM], BF16, tag="kvw", bufs=4)
            for h2 in range(2):
                psk = pp.tile([P, DG], F32, tag="pp")
                nc.tensor.matmul(
                    psk[:], kcat[:], wo[dt][:, h2 * DG:(h2 + 1) * DG],
                    start=True, stop=True,
                )
                if h2 == 0:
                    nc.vector.tensor_copy(kvw_t[:, 0:DG], psk[:])
                else:
                    nc.scalar.copy(kvw_t[:, DG:DM], psk[:])
            kvw.append(kvw_t)

    def emit_proj_q(nch):
        qz = []
        for dt in range(DT):
            ps = pp.tile([P, DG], F32, tag="pp")
            for kc in range(KC):
                nc.tensor.matmul(
                    ps[:],
                    wq[kc][:, dt * P:(dt + 1) * P],
                    xq[kc][:, nch * DG:(nch + 1) * DG],
                    start=(kc == 0), stop=(kc == KC - 1),
                )
            qf = qftp.tile([P, DG], BF16, tag="qft")
            feature_map(ps, qf)
            qz.append(qf)
        return qz

    def emit_zchain(qz):
        # ZpreT (8 heads, 512 n); zra = 1/ZpreT; QzT = QfT * selbcast(zra)
        zp = kvp.tile([8, DG], F32, tag="acc", name="zp")
        for dt in range(DT):
            nc.tensor.matmul(
                zp[:], ks32[:, dt * 8:(dt + 1) * 8], qz[dt][:],
                start=(dt == 0), stop=(dt == DT - 1),
            )
        zra = misc.tile([8, DG], F32R, tag="zra", bufs=2)
        nc.vector.reciprocal(zra[:], zp[:])
        for dt in range(DT):
            zrp = kvp.tile([P, DG], F32, tag="acc", name="zrp")
            nc.tensor.matmul(zrp[:], sel8[dt][:], zra[:],
                             start=True, stop=True)
            nc.vector.tensor_tensor(qz[dt][:], qz[dt][:], zrp[:], Alu.mult)

    def emit_ygemm(nch, qz):
        for jb in range(8):
            yps = pp.tile([P, DG], F32, tag="pp")
            for dt in range(DT):
                nc.tensor.matmul(
                    yps[:],
                    kvw[dt][:, jb * P:(jb + 1) * P],
                    qz[dt][:],
                    start=(dt == 0), stop=(dt == DT - 1),
                )
            yt = ysb.tile([P, DG], BF16, tag="ysb")
            if (jb + nch) % 2 == 0:
                nc.vector.tensor_copy(yt[:], yps[:])
            else:
                nc.scalar.copy(yt[:], yps[:])
            nc.sync.dma_start(
                y_d[jb * P:(jb + 1) * P, nch * DG:(nch + 1) * DG], yt[:])

    prev = None
    for nch in range(NCH):
        qz = emit_proj_q(nch)
        if nch == 0:
            emit_kvw()
        emit_zchain(qz)
        if prev is not None:
            emit_ygemm(*prev)
        prev = (nch, qz)
    emit_ygemm(*prev)


def make_in_maps(q, k, v, w_q, w_k, w_v, w_o):
    bf16 = mybir.dt.np(BF16)
    f32 = np.float32
    q = np.asarray(q, dtype=f32)
    k = np.asarray(k, dtype=f32)
    v = np.asarray(v, dtype=f32)
    w_q = np.asarray(w_q, dtype=f32)
    w_k = np.asarray(w_k, dtype=f32)
    w_v = np.asarray(w_v, dtype=f32)
    w_o = np.asarray(w_o, dtype=f32)
    B = q.shape[0]
    xqT = [q[b].T.astype(bf16) for b in range(B)]
    xkT = [k[b].T.astype(bf16) for b in range(B)]
    xvT = [v[b].T.astype(bf16) for b in range(B)]
    wqT = [w_q[g * DG:(g + 1) * DG, :].T.astype(bf16) for g in range(2)]
    wkT = [w_k[g * DG:(g + 1) * DG, :].T.astype(bf16) for g in range(2)]
    wvT = [w_v[g * DG:(g + 1) * DG, :].T.astype(bf16) for g in range(2)]
    woT = [w_o[:, g * DG:(g + 1) * DG].T.astype(bf16) for g in range(2)]
    sel8 = np.zeros((32, P), dtype=np.float32)
    for dt in range(4):
        sel8[dt * 8 + 2 * dt, 0:64] = 1.0
        sel8[dt * 8 + 2 * dt + 1, 64:128] = 1.0
    in_maps = []
    for c in range(8):
        b, g = c // 2, c % 2
        in_maps.append({
            "xqT": xqT[b], "xkT": xkT[b], "xvT": xvT[b],
            "wqT": wqT[g], "wkT": wkT[g], "wvT": wvT[g], "woT": woT[g],
            "sel8": sel8,
        })
    return in_maps


def kernel(q, k, v, mask, w_q, w_k, w_v, w_o):
    if "nc" not in _CACHE:
        _CACHE["nc"] = build_nc()
    nc = _CACHE["nc"]
    in_maps = make_in_maps(q, k, v, w_q, w_k, w_v, w_o)
    res = run_bass_kernel_spmd(nc, in_maps, list(range(8)))
    _CACHE["last_results"] = res
    B = np.asarray(q).shape[0]
    out = np.empty((B, L, DM), dtype=np.float32)
    for b in range(B):
        y0 = np.asarray(res.results[2 * b]["y"]).astype(np.float32)
        y1 = np.asarray(res.results[2 * b + 1]["y"]).astype(np.float32)
        out[b] = (y0 + y1).T
    return out
